# revision 33
# baseline (speedup 1.0000x reference)
"""Trainium2 Bass kernel for the SOCS lithography simulator.

Reference math (per batch b):
    aerial = sum_k s_k * | cIFFT2( cFFT2(mask_b) * pad_center(kernels[k]) ) |^2
    resist = sigmoid(50*(aerial - 0.225));  printed = (aerial > 0.225)

The padded kernels live in the *frequency* domain with only a 35x35 window of
nonzero coefficients (rows/cols 494:529 of the centered spectrum), so every
field is band-limited to 35x35 frequencies and aerial (a sum of |field|^2) is
band-limited to 69x69.  That turns the whole thing into small dense matmuls:

    Mhat  = A @ x @ A.T          A = rows 494:529 of the centered DFT matrix
    G_k   = Mhat * (sqrt(s_k) * kernels[k])                 [35,35] cplx
    W_k   = G_k @ C.T            C = coarse (stride-8) inverse-DFT samples
    Fc_k  = C @ W_k              fields on the 128x128 coarse grid
    aer_c = sum_k |Fc_k|^2       exact coarse samples of aerial
    aerial = U @ aer_c @ U.T     U real [1024,128] Dirichlet interp (exact)

Optimizations vs the first working version (80.7us -> ~57us):
  * device outputs ONLY aerial (bf16) - resist/printed are cheap, exactly
    reconstructible host-side transforms of aerial (sigmoid / threshold).
    Cuts output HBM traffic 6x.
  * stage 5 runs in bf16 (was f32r): 4x faster matmuls, half the U DMA.
  * stage 2a elementwise in bf16 on 99-row Re/Im stacks (DVE 2x 16-bit mode),
    interleaved in halves so stage-2c matmuls start early.
  * stage 2c does 2 kernels per matmul via [99,99] pair-block stationaries
    (29-col gaps keep partition slices 0/64-aligned); stage 2d uses 99-row
    stacked stationaries: 2 matmuls per 4-kernel group instead of 4.
  * stage 1b is one fused accumulation chain ([Ar|gap|Ai] stationary).
  * |F|^2 squares are whole-psum-tile scalar ACTIVATE ops; intensity sums are
    a bf16 tree on vector with group-0:3 folds hoisted off the critical tail.
  * x is loaded y-interleaved (partition p holds DRAM rows 8p+2c..8p+2c+1) and
    uht cols are pre-permuted so input/output DMA descriptors cover 4KB runs;
    DMA issue is spread across sync/scalar/gpsimd queues.
  * engine balance: gpsimd only does DMA issue + one hidden tree add (its
    elementwise ops are 3-4x slower than DVE); psum->sbuf copies split
    between scalar and vector (each reads at most one PSUM operand).

Hardware rules learned (cost a debug cycle each, do not regress):
  * a start=True matmul clears has_written bits for its whole PSUM bank ->
    concurrent accumulation chains need one bank each; single-shot
    (start+stop) matmuls may share a bank.
  * matmul PSUM output regions must not cross a 2KB bank boundary.
  * engine AP partition offsets must be multiples of 32 (hence the
    0:35 / 64:99 "99-row stack" layout used everywhere).
  * GPSIMD cannot read PSUM; DVE/ACT can read at most one PSUM operand.
  * collective_compute has a ~10us floor per op on this fabric (first one
    ~40us) - pair-wise k/y-splits via collectives do not pay off here.

Sharding: 8 cores; core c handles batch c//2 and output row-half c%2.
Each core runs stages 1-4 for its batch and half of stage 5. No collectives.

Self-contained: shapes/constants hardcoded, no sibling imports.
"""

import os

import numpy as np

N = 1024
B, K, HK = 4, 24, 35
PT = (N - HK) // 2          # 494
NC = 72                     # coarse grid samples (stride 14; >= 69 needed)
NF = 2 * HK - 1             # 69 product frequencies
RESIST_THRESHOLD = 0.225
RESIST_STEEPNESS = 50.0


# ---------------------------------------------------------------- host matrices
def _host_matrices():
    u = np.arange(HK)[:, None]          # 0..34  (centered freq u-18)
    y = np.arange(N)[None, :]
    A = np.exp(-2j * np.pi * ((u + PT - N // 2) * (y - N // 2)) / N)  # [35,1024]
    # coarse sample points: 72 uniform stride-14 positions (>= 69 needed for
    # the 69-frequency band-limited aerial; cond(V) ~ 6 at this choice)
    ym = 14 * np.arange(NC)
    Cs = np.exp(2j * np.pi * ((np.arange(HK)[None, :] - 18)
                              * (ym[:, None] - 512)) / N) / N         # [72,35]
    f = np.arange(-(NF // 2), NF // 2 + 1)
    V = np.exp(2j * np.pi * (f[None, :] * (ym[:, None] - 512)) / N)   # [72,69]
    E = np.exp(2j * np.pi * (f[None, :]
                             * (np.arange(N)[:, None] - 512)) / N)    # [1024,69]
    U = np.ascontiguousarray((E @ np.linalg.pinv(V)).real)            # [1024,72]

    atp = np.empty((N, 2 * HK), np.float32)          # [1024, 70]  A^T packed
    atp[:, :HK] = A.real.T
    atp[:, HK:] = A.imag.T
    ctr = np.ascontiguousarray(Cs.real.T, np.float32)   # [35,72] Ctr[q,m]=ReC[m,q]
    cti = np.ascontiguousarray(Cs.imag.T, np.float32)
    # ctp99: stacked rhs for stage 2c (contract Re/Im of G in one matmul)
    ctp99 = np.zeros((99, 2 * NC), np.float32)
    ctp99[0:35] = np.concatenate([ctr, cti], axis=1)        # top: [ctr | cti]
    ctp99[64:99] = np.concatenate([-cti, ctr], axis=1)      # bot: [-cti | ctr]
    # cc99: stacked stationary for stage 2d. col block 0: Re out, 1: Im out
    cc99 = np.zeros((99, 2 * NC), np.float32)
    cc99[0:35, 0:NC] = ctr
    cc99[64:99, 0:NC] = -cti
    cc99[0:35, NC:2 * NC] = cti
    cc99[64:99, NC:2 * NC] = ctr
    ut = np.ascontiguousarray(U.T, np.float32)          # [72,1024]
    return atp, ctp99, cc99, ut, U.astype(np.float32)


# ---------------------------------------------------------------- bass program
def _build_program():
    import concourse.bass as bass
    import concourse.mybir as mybir
    import concourse.tile as tile
    from concourse import bacc

    f32 = mybir.dt.float32
    bf16 = mybir.dt.bfloat16
    AF = mybir.ActivationFunctionType

    nc = bacc.Bacc("TRN2", target_bir_lowering=False, debug=False)

    x_d = nc.dram_tensor("x", [N, N], bf16, kind="ExternalInput")
    # atp cols 0:560 y-interleaved (stage 1), 560:1352 j-chunked 99-col
    # stacks [Ar | gap | Ai] (stage 1b single-chain stationary)
    atp_d = nc.dram_tensor("atp", [128, 1352], bf16, kind="ExternalInput")
    # kri: 99-row stacks (rows 0:35 / 64:99) with 12 pair-blocks of 99 cols;
    # cols 0:1188 multiply M_r (Kr-; Ki-stack), cols 1188:2376 multiply M_i
    kri_d = nc.dram_tensor("kri", [99, 2 * 12 * 99], bf16, kind="ExternalInput")
    # cc = [ctp99 (144) | cc99r (72) | cc99i (72)]  [99, 288]
    cc_d = nc.dram_tensor("cc", [99, 288], bf16, kind="ExternalInput")
    # uc = [uht_h | ut]  [72, 1536]
    uc_d = nc.dram_tensor("uc", [NC, 1536], bf16, kind="ExternalInput")

    aerial_d = nc.dram_tensor("aerial", [512, N], bf16, kind="ExternalOutput")

    with tile.TileContext(nc) as tc:
        with (
            tc.tile_pool(name="const", bufs=1) as cpool,
            tc.tile_pool(name="xin", bufs=8) as xpool,
            tc.tile_pool(name="work", bufs=1) as wpool,
            tc.tile_pool(name="scr", bufs=2) as spool,
            tc.tile_pool(name="sq", bufs=6) as sqpool,
            tc.tile_pool(name="outp", bufs=3) as opool,
        ):
            # ---- input DMAs: x chunks on sync queue, consts on gpsimd ----
            # x chunk c holds DRAM rows 8p+2c, 8p+2c+1 on partition p -> the
            # two rows are adjacent in DRAM = 4KB descriptor runs
            x_sb = [xpool.tile([128, 2, N], bf16, tag="x", name=f"x{i}")
                    for i in range(4)]
            xv = x_d.ap().rearrange("(p r) j -> p r j", p=128)
            x_qs = [nc.sync, nc.scalar, nc.sync, nc.scalar]
            for c in range(4):
                x_qs[c].dma_start(x_sb[c][:], xv[:, 2 * c:2 * c + 2, :])

            atp_sb = cpool.tile([128, 1352], bf16)
            nc.gpsimd.dma_start(atp_sb[:], atp_d[:, :])
            kri_sb = cpool.tile([99, 2 * 12 * 99], bf16)
            nc.gpsimd.dma_start(kri_sb[:], kri_d[:, :])
            cc_sb = cpool.tile([99, 288], bf16)
            nc.gpsimd.dma_start(cc_sb[:], cc_d[:, :])
            uc_sb = cpool.tile([NC, 1536], bf16)
            nc.gpsimd.dma_start(uc_sb[:], uc_d[:, :])

            # early memsets (no input deps; keep off the critical path)
            mhat99_r = wpool.tile([99, 128], bf16)
            mhat99_i = wpool.tile([99, 128], bf16)
            gt = wpool.tile([99, 12 * 99], bf16)
            w99 = wpool.tile([99, K * NC], bf16)          # [99, 1728]
            nc.vector.memset(mhat99_r[:], 0.0)
            nc.vector.memset(mhat99_i[:], 0.0)
            nc.vector.memset(gt[32:64, :], 0.0)
            nc.gpsimd.memset(w99[32:64, :], 0.0)

            ctp99 = cc_sb[:, 0:144]
            cc99r = cc_sb[:, 144:216]
            cc99i = cc_sb[:, 216:288]
            uht = uc_sb[:, 0:512]
            ut = uc_sb[:, 512:1536]

            # ---- stage 1: P1T[j,u] = sum_y x[y,j] * atp[y,u] ----
            # NOTE: a chain's start=True matmul clears has_written bits for the
            # whole PSUM bank, so concurrent accumulation chains must live in
            # separate banks -> one tile (bank) per chain.
            p1t_sb = wpool.tile([128, 8 * 2 * HK], bf16)      # [128, 560]
            with tc.tile_pool(name="p1ps", bufs=8, space=bass.MemorySpace.PSUM) as p1ps:
                p1t_ps = [p1ps.tile([128, 2 * HK], f32, tag="p1t", name=f"p1t{i}")
                          for i in range(8)]
                for c in range(4):
                    for r01 in range(2):
                        for jc in range(8):
                            nc.tensor.matmul(
                                p1t_ps[jc][:, :],
                                x_sb[c][:, r01, jc * 128:(jc + 1) * 128],
                                atp_sb[:, (2 * c + r01) * 70:(2 * c + r01 + 1) * 70],
                                start=(c == 0 and r01 == 0),
                                stop=(c == 3 and r01 == 1),
                            )
                for jc in range(8):
                    if jc % 2 == 0:
                        nc.scalar.copy(p1t_sb[:, jc * 70:(jc + 1) * 70], p1t_ps[jc][:, :])
                    else:
                        nc.vector.tensor_copy(p1t_sb[:, jc * 70:(jc + 1) * 70],
                                              p1t_ps[jc][:, :])

            # ---- stage 1b: MhatT = A @ P1^T (contract over j), 99-row stack ----
            # mhat99_* rows 0:35 and 64:99 (and cols 0:35, 64:99) hold MhatT;
            # the 29-row/col gaps keep every partition slice 0/64-aligned.
            with tc.tile_pool(name="m4ps", bufs=1, space=bass.MemorySpace.PSUM) as m4ps:
                # single chain: stationary [Ar | gap | Ai] gives rows 0:35 =
                # Ar@[P1r|P1i], rows 64:99 = Ai@[P1r|P1i]
                m4 = m4ps.tile([99, 2 * HK], f32)
                for jc in range(8):
                    nc.tensor.matmul(m4[:, :], atp_sb[:, 560 + jc * 99:560 + (jc + 1) * 99],
                                     p1t_sb[:, jc * 70:(jc + 1) * 70],
                                     start=(jc == 0), stop=(jc == 7))
                m4b_sb = wpool.tile([HK, 2 * HK], f32)
                nc.scalar.copy(m4b_sb[:], m4[64:99, :])
                # MhatT_r = ArP1r - AiP1i ; MhatT_i = ArP1i + AiP1r; write the
                # cols-{0:35,64:99} pair per op via a strided 3D view
                cview = lambda t, pq: t[pq:pq + HK, :].rearrange(
                    "p (c u) -> p c u", c=2)[:, :, 0:HK]   # cols {0:35, 64:99}
                bcast = lambda ap: ap.unsqueeze(1).broadcast_to([HK, 2, HK])
                for pq in (0, 64):
                    nc.vector.tensor_sub(cview(mhat99_r, pq),
                                         bcast(m4[0:HK, 0:HK]),
                                         bcast(m4b_sb[:, HK:2 * HK]))
                    nc.vector.tensor_add(cview(mhat99_i, pq),
                                         bcast(m4[0:HK, HK:2 * HK]),
                                         bcast(m4b_sb[:, 0:HK]))

            # ---- stage 2a: Gt = MhatT .* Kt (complex), 99-row/99-col blocks ----
            t1 = spool.tile([99, 12 * 99], bf16, tag="t", name="t1")
            t2 = spool.tile([99, 12 * 99], bf16, tag="t", name="t2")
            r3 = lambda ap, k: ap.rearrange("q (k p) -> q k p", k=k)
            mr_b = mhat99_r[:, 0:99].unsqueeze(1).broadcast_to([99, 12, 99])
            mi_b = mhat99_i[:, 0:99].unsqueeze(1).broadcast_to([99, 12, 99])
            # t1 = M99r * [kR;kI] on vector; t2 = M99i * [kI;kR] gpsimd/vector
            for half in range(2):
                c0, c1 = half * 594, (half + 1) * 594
                mr_b6 = mhat99_r[:, 0:99].unsqueeze(1).broadcast_to([99, 6, 99])
                mi_b6 = mhat99_i[:, 0:99].unsqueeze(1).broadcast_to([99, 6, 99])
                nc.vector.tensor_mul(r3(t1[:, c0:c1], 6), mr_b6,
                                     r3(kri_sb[:, c0:c1], 6))
                nc.vector.tensor_mul(r3(t2[:, c0:c1], 6), mi_b6,
                                     r3(kri_sb[:, 1188 + c0:1188 + c1], 6))
                nc.vector.tensor_sub(gt[0:HK, c0:c1], t1[0:HK, c0:c1], t2[0:HK, c0:c1])
                nc.vector.tensor_add(gt[64:99, c0:c1], t1[64:99, c0:c1], t2[64:99, c0:c1])

            # ---- stage 2c: W pairs; w99 = [Wr; 0; Wi] [99, 1728] ----
            # col layout: pair p low-k at 72p (0:864), high-k at 864+72p
            sq = [sqpool.tile([72, 944], bf16, tag="sq", name=f"sq{g}")
                  for g in range(4)]
            with (
                tc.tile_pool(name="wps", bufs=2, space=bass.MemorySpace.PSUM) as wps,
                tc.tile_pool(name="fps", bufs=2, space=bass.MemorySpace.PSUM) as fps,
            ):
                offs = (0, 144, 288, 512, 656, 800)    # 3 pair-slots per bank
                for t6 in range(2):                    # 6 pairs (12 kernels) / tile
                    wp = wps.tile([99, 1024], f32)
                    for j in range(6):
                        pr = t6 * 6 + j
                        nc.tensor.matmul(wp[:, offs[j]:offs[j] + 144],
                                         gt[:, pr * 99:(pr + 1) * 99],
                                         ctp99, start=True, stop=True)
                    # wp rows: 0:35 = W_lo, 64:99 = W_hi; [99, 2, 3, 144] view
                    wpv = wp[:].rearrange("q (z c) -> q z c", z=2)[
                        :, :, 0:432].rearrange("q z (j m) -> q z j m", j=3)
                    cl = slice(t6 * 432, (t6 + 1) * 432)
                    ch = slice(864 + t6 * 432, 864 + (t6 + 1) * 432)
                    r2 = lambda ap: ap.rearrange("q (z j m) -> q z j m", z=2, j=3)
                    nc.vector.tensor_copy(r2(w99[0:HK, cl]), wpv[0:HK, :, :, 0:72])
                    nc.vector.tensor_copy(r2(w99[64:99, cl]), wpv[0:HK, :, :, 72:144])
                    nc.scalar.copy(r2(w99[0:HK, ch]), wpv[64:99, :, :, 0:72])
                    nc.vector.tensor_copy(r2(w99[64:99, ch]), wpv[64:99, :, :, 72:144])

                # ---- stage 2d: F groups (6 kernels) + squares (Re/Im spans) ----
                for rnd in range(2):
                    fpa = fps.tile([72, 1024], f32, tag="fp", name="fpa")
                    fpb = fps.tile([72, 1024], f32, tag="fp", name="fpb")
                    ga, gb = 2 * rnd, 2 * rnd + 1
                    nc.tensor.matmul(fpa[:, 0:432], cc99r,
                                     w99[:, ga * 432:(ga + 1) * 432],
                                     start=True, stop=True)
                    nc.tensor.matmul(fpa[:, 432:512], cc99r, w99[:, 0:80],
                                     start=True, stop=True)
                    nc.tensor.matmul(fpa[:, 512:944], cc99i,
                                     w99[:, ga * 432:(ga + 1) * 432],
                                     start=True, stop=True)
                    nc.scalar.activation(sq[ga][:], fpa[:, 0:944], AF.Square)
                    nc.tensor.matmul(fpb[:, 0:432], cc99r,
                                     w99[:, gb * 432:(gb + 1) * 432],
                                     start=True, stop=True)
                    nc.tensor.matmul(fpb[:, 432:512], cc99r, w99[:, 0:80],
                                     start=True, stop=True)
                    nc.tensor.matmul(fpb[:, 512:944], cc99i,
                                     w99[:, gb * 432:(gb + 1) * 432],
                                     start=True, stop=True)
                    nc.scalar.activation(sq[gb][:], fpb[:, 0:944], AF.Square)

            # ---- intensity sum: p-chain (groups 0,1) folds early; q-chain
            # (groups 2,3) trails the last square ----
            pq_out = []
            for ch, (gx, gy) in enumerate(((0, 1), (2, 3))):
                vr = spool.tile([72, 432], bf16, tag=f"v{ch}", name=f"vr{ch}")
                vi = spool.tile([72, 432], bf16, tag=f"v{ch}", name=f"vi{ch}")
                nc.vector.tensor_add(vr[:], sq[gx][:, 0:432], sq[gy][:, 0:432])
                nc.vector.tensor_add(vi[:], sq[gx][:, 512:944], sq[gy][:, 512:944])
                fa = wpool.tile([72, 432], f32, tag=f"fa{ch}", name=f"fa{ch}")
                fb = wpool.tile([72, 216], f32, tag=f"fb{ch}", name=f"fb{ch}")
                fc = wpool.tile([72, 72], f32, tag=f"fc{ch}", name=f"fc{ch}")
                fd = wpool.tile([72, 72], bf16, tag=f"fd{ch}", name=f"fd{ch}")
                nc.vector.tensor_add(fa[:], vr[:], vi[:])
                nc.vector.tensor_add(fb[:], fa[:, 0:216], fa[:, 216:432])
                nc.vector.tensor_add(fc[:], fb[:, 0:72], fb[:, 72:144])
                nc.vector.tensor_add(fd[:], fc[:], fb[:, 144:216])
                pq_out.append(fd)

            # ---- stage 5: aerial_half = U_h @ aer_c @ U^T (bf16 matmuls) ----
            z_sb = wpool.tile([72, 512], bf16)
            with tc.tile_pool(name="zps", bufs=1, space=bass.MemorySpace.PSUM) as zps:
                zp = zps.tile([72, 512], f32)
                nc.tensor.matmul(zp[:], pq_out[0][:], uht, start=True, stop=False)
                nc.tensor.matmul(zp[:], pq_out[1][:], uht, start=False, stop=True)
                nc.scalar.copy(z_sb[:, 0:256], zp[:, 0:256])
                nc.vector.tensor_copy(z_sb[:, 256:512], zp[:, 256:512])

            # uht cols are host-permuted: z col 128*(2*tau+s)+p holds output
            # row 256*tau + 2p + s, so partition p carries 2 adjacent DRAM rows
            # per 256-row tile -> 4KB output descriptor runs
            with tc.tile_pool(name="aps", bufs=2, space=bass.MemorySpace.PSUM) as aps:
                for tau in range(2):
                    aer_sb = opool.tile([128, 2 * N], bf16, tag="out", name="aer_sb")
                    for s in range(2):
                        ap_t = aps.tile([128, N], f32)
                        zc = 256 * tau + 128 * s
                        nc.tensor.matmul(ap_t[:, 0:512],
                                         z_sb[:, zc:zc + 128],
                                         ut[:, 0:512], start=True, stop=True)
                        nc.tensor.matmul(ap_t[:, 512:1024],
                                         z_sb[:, zc:zc + 128],
                                         ut[:, 512:1024], start=True, stop=True)
                        if s == 0:
                            nc.scalar.copy(aer_sb[:, s * N:(s + 1) * N], ap_t[:])
                        else:
                            nc.vector.tensor_copy(aer_sb[:, s * N:(s + 1) * N],
                                                  ap_t[:])
                    oq = nc.sync if tau == 0 else nc.gpsimd
                    oq.dma_start(
                        aerial_d[256 * tau:256 * (tau + 1), :].rearrange(
                            "(p s) y -> p (s y)", p=128),
                        aer_sb[:])

    nc.compile()
    return nc


_CACHE = {}


def _get_program():
    if "nc" not in _CACHE:
        _CACHE["nc"] = _build_program()
    return _CACHE["nc"]


def _prep_inputs(mask, kernels, scales):
    import ml_dtypes
    bf = ml_dtypes.bfloat16

    atp, ctp99, cc99, ut, U = _host_matrices()

    kers = kernels.astype(np.complex128) * np.sqrt(scales.astype(np.float64))[:, None, None]
    ktR = np.ascontiguousarray(
        kers.real.astype(np.float32).transpose(2, 0, 1).reshape(HK, K * HK))
    ktI = np.ascontiguousarray(
        kers.imag.astype(np.float32).transpose(2, 0, 1).reshape(HK, K * HK))
    # 99-row / 99-col pair-block layout: block p holds kernels (2p, 2p+1) at
    # cols 0:35 / 64:99; rows 0:35 multiply M (kA top), rows 64:99 the swap.
    kri = np.zeros((99, 2 * 12 * 99), np.float32)
    for p in range(12):
        for side, k in ((0, 2 * p), (64, 2 * p + 1)):
            c = p * 99 + side
            kri[0:HK, c:c + HK] = ktR[:, k * HK:(k + 1) * HK]        # t1 top: Kr
            kri[64:99, c:c + HK] = ktI[:, k * HK:(k + 1) * HK]       # t1 bot: Ki
            kri[0:HK, 1188 + c:1188 + c + HK] = ktI[:, k * HK:(k + 1) * HK]
            kri[64:99, 1188 + c:1188 + c + HK] = ktR[:, k * HK:(k + 1) * HK]
    kri = kri.astype(bf)
    # atp: cols 0:560 y-interleaved (atp[8p+r, u], stage 1); cols 560:1352
    # j-chunked 99-col stacks [Ar(35) | zeros(29) | Ai(35)] (stage 1b)
    atp_y = atp.reshape(128, 8 * 2 * HK)                         # [128, 560]
    atp_j = atp.reshape(8, 128, 2 * HK).transpose(1, 0, 2)       # [128, 8, 70]
    atp99 = np.zeros((128, 8, 99), np.float32)
    atp99[:, :, 0:HK] = atp_j[:, :, 0:HK]
    atp99[:, :, 64:99] = atp_j[:, :, HK:2 * HK]
    atp = np.ascontiguousarray(
        np.concatenate([atp_y, atp99.reshape(128, 792)], axis=1))
    cc = np.concatenate([ctp99, cc99], axis=1).astype(bf)      # [99, 512]
    # uht cols permuted so stage-5b's z col 128*(2*tau+s)+p holds output row
    # 256*tau + 2p + s (2 adjacent DRAM rows per partition in the output DMA)
    cidx = np.arange(512)
    rperm = 256 * (cidx // 256) + 2 * (cidx % 128) + ((cidx % 256) // 128)
    uh = [np.ascontiguousarray(U[h * 512:(h + 1) * 512, :].T[:, rperm])
          for h in range(2)]
    uc = [np.concatenate([uh[h], ut], axis=1).astype(bf) for h in range(2)]
    atp_bf = atp.astype(bf)
    mask_bf = np.asarray(mask, np.float32).astype(bf)
    return mask_bf, atp_bf, kri, cc, uc


# ---------------------------------------------------------------- entry point
def kernel(mask, kernels, kernels_ct, scales):
    """Full inputs in, full outputs out.  Shards over 8 NeuronCores internally."""
    from concourse.bass_utils import run_bass_kernel_spmd

    kernels = np.asarray(kernels, np.complex64)
    scales = np.asarray(scales, np.float32)
    mask_bf, atp_bf, kri, cc, uc = _prep_inputs(mask, kernels, scales)

    nc = _get_program()
    in_maps = []
    for c in range(8):
        b, h = c // 2, c % 2
        in_maps.append({
            "x": mask_bf[b],
            "atp": atp_bf,
            "kri": kri,
            "cc": cc,
            "uc": uc[h],
        })

    trace = bool(int(os.environ.get("BASS_KERNEL_TRACE", "0")))
    res = run_bass_kernel_spmd(nc, in_maps, core_ids=list(range(8)), trace=trace)
    _CACHE["last_results"] = res

    aerial = np.empty((B, N, N), np.float32)
    for c in range(8):
        b, h = c // 2, c % 2
        aerial[b, h * 512:(h + 1) * 512, :] = \
            np.asarray(res.results[c]["aerial"]).astype(np.float32)
    resist = (1.0 / (1.0 + np.exp(
        -RESIST_STEEPNESS * (aerial.astype(np.float64) - RESIST_THRESHOLD)
    ))).astype(np.float32)
    printed = (aerial > RESIST_THRESHOLD).astype(np.float32)
    return aerial, resist, printed


# revision 34
# speedup vs baseline: 1.0305x; 1.0305x over previous
"""Trainium2 Bass kernel for the SOCS lithography simulator.

Reference math (per batch b):
    aerial = sum_k s_k * | cIFFT2( cFFT2(mask_b) * pad_center(kernels[k]) ) |^2
    resist = sigmoid(50*(aerial - 0.225));  printed = (aerial > 0.225)

The padded kernels live in the *frequency* domain with only a 35x35 window of
nonzero coefficients (rows/cols 494:529 of the centered spectrum), so every
field is band-limited to 35x35 frequencies and aerial (a sum of |field|^2) is
band-limited to 69x69.  That turns the whole thing into small dense matmuls:

    Mhat  = A @ x @ A.T          A = rows 494:529 of the centered DFT matrix
    G_k   = Mhat * (sqrt(s_k) * kernels[k])                 [35,35] cplx
    W_k   = G_k @ C.T            C = coarse (stride-8) inverse-DFT samples
    Fc_k  = C @ W_k              fields on the 128x128 coarse grid
    aer_c = sum_k |Fc_k|^2       exact coarse samples of aerial
    aerial = U @ aer_c @ U.T     U real [1024,128] Dirichlet interp (exact)

Optimizations vs the first working version (80.7us -> ~57us):
  * device outputs ONLY aerial (bf16) - resist/printed are cheap, exactly
    reconstructible host-side transforms of aerial (sigmoid / threshold).
    Cuts output HBM traffic 6x.
  * stage 5 runs in bf16 (was f32r): 4x faster matmuls, half the U DMA.
  * stage 2a elementwise in bf16 on 99-row Re/Im stacks (DVE 2x 16-bit mode),
    interleaved in halves so stage-2c matmuls start early.
  * stage 2c does 2 kernels per matmul via [99,99] pair-block stationaries
    (29-col gaps keep partition slices 0/64-aligned); stage 2d uses 99-row
    stacked stationaries: 2 matmuls per 4-kernel group instead of 4.
  * stage 1b is one fused accumulation chain ([Ar|gap|Ai] stationary).
  * |F|^2 squares are whole-psum-tile scalar ACTIVATE ops; intensity sums are
    a bf16 tree on vector with group-0:3 folds hoisted off the critical tail.
  * x is loaded y-interleaved (partition p holds DRAM rows 8p+2c..8p+2c+1) and
    uht cols are pre-permuted so input/output DMA descriptors cover 4KB runs;
    DMA issue is spread across sync/scalar/gpsimd queues.
  * engine balance: gpsimd only does DMA issue + one hidden tree add (its
    elementwise ops are 3-4x slower than DVE); psum->sbuf copies split
    between scalar and vector (each reads at most one PSUM operand).

Hardware rules learned (cost a debug cycle each, do not regress):
  * a start=True matmul clears has_written bits for its whole PSUM bank ->
    concurrent accumulation chains need one bank each; single-shot
    (start+stop) matmuls may share a bank.
  * matmul PSUM output regions must not cross a 2KB bank boundary.
  * engine AP partition offsets must be multiples of 32 (hence the
    0:35 / 64:99 "99-row stack" layout used everywhere).
  * GPSIMD cannot read PSUM; DVE/ACT can read at most one PSUM operand.
  * collective_compute has a ~10us floor per op on this fabric (first one
    ~40us) - pair-wise k/y-splits via collectives do not pay off here.

Sharding: 8 cores; core c handles batch c//2 and output row-half c%2.
Each core runs stages 1-4 for its batch and half of stage 5. No collectives.

Self-contained: shapes/constants hardcoded, no sibling imports.
"""

import os

import numpy as np

N = 1024
B, K, HK = 4, 24, 35
PT = (N - HK) // 2          # 494
NC = 72                     # coarse grid samples (stride 14; >= 69 needed)
NF = 2 * HK - 1             # 69 product frequencies
RESIST_THRESHOLD = 0.225
RESIST_STEEPNESS = 50.0


# ---------------------------------------------------------------- host matrices
def _host_matrices():
    u = np.arange(HK)[:, None]          # 0..34  (centered freq u-18)
    y = np.arange(N)[None, :]
    A = np.exp(-2j * np.pi * ((u + PT - N // 2) * (y - N // 2)) / N)  # [35,1024]
    # coarse sample points: 72 uniform stride-14 positions (>= 69 needed for
    # the 69-frequency band-limited aerial; cond(V) ~ 6 at this choice)
    ym = 14 * np.arange(NC)
    Cs = np.exp(2j * np.pi * ((np.arange(HK)[None, :] - 18)
                              * (ym[:, None] - 512)) / N) / N         # [72,35]
    f = np.arange(-(NF // 2), NF // 2 + 1)
    V = np.exp(2j * np.pi * (f[None, :] * (ym[:, None] - 512)) / N)   # [72,69]
    E = np.exp(2j * np.pi * (f[None, :]
                             * (np.arange(N)[:, None] - 512)) / N)    # [1024,69]
    U = np.ascontiguousarray((E @ np.linalg.pinv(V)).real)            # [1024,72]

    atp = np.empty((N, 2 * HK), np.float32)          # [1024, 70]  A^T packed
    atp[:, :HK] = A.real.T
    atp[:, HK:] = A.imag.T
    ctr = np.ascontiguousarray(Cs.real.T, np.float32)   # [35,72] Ctr[q,m]=ReC[m,q]
    cti = np.ascontiguousarray(Cs.imag.T, np.float32)
    # ctp99: stacked rhs for stage 2c (contract Re/Im of G in one matmul)
    ctp99 = np.zeros((99, 2 * NC), np.float32)
    ctp99[0:35] = np.concatenate([ctr, cti], axis=1)        # top: [ctr | cti]
    ctp99[64:99] = np.concatenate([-cti, ctr], axis=1)      # bot: [-cti | ctr]
    # cc99: stacked stationary for stage 2d. col block 0: Re out, 1: Im out
    cc99 = np.zeros((99, 2 * NC), np.float32)
    cc99[0:35, 0:NC] = ctr
    cc99[64:99, 0:NC] = -cti
    cc99[0:35, NC:2 * NC] = cti
    cc99[64:99, NC:2 * NC] = ctr
    ut = np.ascontiguousarray(U.T, np.float32)          # [72,1024]
    return atp, ctp99, cc99, ut, U.astype(np.float32)


# ---------------------------------------------------------------- bass program
def _build_program():
    import concourse.bass as bass
    import concourse.mybir as mybir
    import concourse.tile as tile
    from concourse import bacc

    f32 = mybir.dt.float32
    bf16 = mybir.dt.bfloat16
    AF = mybir.ActivationFunctionType

    nc = bacc.Bacc("TRN2", target_bir_lowering=False, debug=False)

    x_d = nc.dram_tensor("x", [N, N], bf16, kind="ExternalInput")
    # atp cols 0:560 y-interleaved (stage 1), 560:1352 j-chunked 99-col
    # stacks [Ar | gap | Ai] (stage 1b single-chain stationary)
    atp_d = nc.dram_tensor("atp", [128, 1352], bf16, kind="ExternalInput")
    # kri: 99-row stacks (rows 0:35 / 64:99) with 12 pair-blocks of 99 cols;
    # cols 0:1188 multiply M_r (Kr-; Ki-stack), cols 1188:2376 multiply M_i
    kri_d = nc.dram_tensor("kri", [99, 2 * 12 * 99], bf16, kind="ExternalInput")
    # cc = [ctp99 (144) | cc99r (72) | cc99i (72)]  [99, 288]
    cc_d = nc.dram_tensor("cc", [99, 288], bf16, kind="ExternalInput")
    # uc = [uht_h | ut]  [72, 1536]
    uc_d = nc.dram_tensor("uc", [NC, 1536], bf16, kind="ExternalInput")

    aerial_d = nc.dram_tensor("aerial", [512, N], bf16, kind="ExternalOutput")

    with tile.TileContext(nc) as tc:
        with (
            tc.tile_pool(name="const", bufs=1) as cpool,
            tc.tile_pool(name="xin", bufs=8) as xpool,
            tc.tile_pool(name="work", bufs=1) as wpool,
            tc.tile_pool(name="scr", bufs=2) as spool,
            tc.tile_pool(name="sq", bufs=6) as sqpool,
            tc.tile_pool(name="outp", bufs=3) as opool,
        ):
            # ---- input DMAs: x chunks on sync queue, consts on gpsimd ----
            # x chunk c holds DRAM rows 8p+2c, 8p+2c+1 on partition p -> the
            # two rows are adjacent in DRAM = 4KB descriptor runs
            x_sb = [xpool.tile([128, 2, N], bf16, tag="x", name=f"x{i}")
                    for i in range(4)]
            xv = x_d.ap().rearrange("(p r) j -> p r j", p=128)
            x_qs = [nc.sync, nc.scalar, nc.sync, nc.scalar]
            for c in range(4):
                x_qs[c].dma_start(x_sb[c][:], xv[:, 2 * c:2 * c + 2, :])

            atp_sb = cpool.tile([128, 1352], bf16)
            nc.gpsimd.dma_start(atp_sb[:], atp_d[:, :])
            kri_sb = cpool.tile([99, 2 * 12 * 99], bf16)
            nc.gpsimd.dma_start(kri_sb[:], kri_d[:, :])
            cc_sb = cpool.tile([99, 288], bf16)
            nc.gpsimd.dma_start(cc_sb[:], cc_d[:, :])
            uc_sb = cpool.tile([NC, 1536], bf16)
            nc.gpsimd.dma_start(uc_sb[:], uc_d[:, :])

            # early memsets (no input deps; keep off the critical path)
            mhat99_r = wpool.tile([99, 128], bf16)
            mhat99_i = wpool.tile([99, 128], bf16)
            gt = wpool.tile([99, 12 * 99], bf16)
            w99 = wpool.tile([99, K * NC], bf16)          # [99, 1728]
            nc.vector.memset(mhat99_r[:], 0.0)
            nc.vector.memset(mhat99_i[:], 0.0)
            nc.vector.memset(gt[32:64, :], 0.0)
            nc.gpsimd.memset(w99[32:64, :], 0.0)

            ctp99 = cc_sb[:, 0:144]
            cc99r = cc_sb[:, 144:216]
            cc99i = cc_sb[:, 216:288]
            uht = uc_sb[:, 0:512]
            ut = uc_sb[:, 512:1536]

            # ---- stage 1: P1T[j,u] = sum_y x[y,j] * atp[y,u] ----
            # NOTE: a chain's start=True matmul clears has_written bits for the
            # whole PSUM bank, so concurrent accumulation chains must live in
            # separate banks -> one tile (bank) per chain.
            p1t_sb = wpool.tile([128, 8 * 2 * HK], bf16)      # [128, 560]
            with tc.tile_pool(name="p1ps", bufs=8, space=bass.MemorySpace.PSUM) as p1ps:
                p1t_ps = [p1ps.tile([128, 2 * HK], f32, tag="p1t", name=f"p1t{i}")
                          for i in range(8)]
                for c in range(4):
                    for r01 in range(2):
                        for jc in range(8):
                            nc.tensor.matmul(
                                p1t_ps[jc][:, :],
                                x_sb[c][:, r01, jc * 128:(jc + 1) * 128],
                                atp_sb[:, (2 * c + r01) * 70:(2 * c + r01 + 1) * 70],
                                start=(c == 0 and r01 == 0),
                                stop=(c == 3 and r01 == 1),
                            )
                for jc in range(8):
                    if jc % 2 == 0:
                        nc.scalar.copy(p1t_sb[:, jc * 70:(jc + 1) * 70], p1t_ps[jc][:, :])
                    else:
                        nc.vector.tensor_copy(p1t_sb[:, jc * 70:(jc + 1) * 70],
                                              p1t_ps[jc][:, :])

            # ---- stage 1b: MhatT = A @ P1^T (contract over j), 99-row stack ----
            # mhat99_* rows 0:35 and 64:99 (and cols 0:35, 64:99) hold MhatT;
            # the 29-row/col gaps keep every partition slice 0/64-aligned.
            with tc.tile_pool(name="m4ps", bufs=1, space=bass.MemorySpace.PSUM) as m4ps:
                # single chain: stationary [Ar | gap | Ai] gives rows 0:35 =
                # Ar@[P1r|P1i], rows 64:99 = Ai@[P1r|P1i]
                m4 = m4ps.tile([99, 2 * HK], f32)
                for jc in range(8):
                    nc.tensor.matmul(m4[:, :], atp_sb[:, 560 + jc * 99:560 + (jc + 1) * 99],
                                     p1t_sb[:, jc * 70:(jc + 1) * 70],
                                     start=(jc == 0), stop=(jc == 7))
                m4b_sb = wpool.tile([HK, 2 * HK], f32)
                nc.scalar.copy(m4b_sb[:], m4[64:99, :])
                # MhatT_r = ArP1r - AiP1i ; MhatT_i = ArP1i + AiP1r; write the
                # cols-{0:35,64:99} pair per op via a strided 3D view
                cview = lambda t, pq: t[pq:pq + HK, :].rearrange(
                    "p (c u) -> p c u", c=2)[:, :, 0:HK]   # cols {0:35, 64:99}
                bcast = lambda ap: ap.unsqueeze(1).broadcast_to([HK, 2, HK])
                for pq in (0, 64):
                    nc.vector.tensor_sub(cview(mhat99_r, pq),
                                         bcast(m4[0:HK, 0:HK]),
                                         bcast(m4b_sb[:, HK:2 * HK]))
                    nc.vector.tensor_add(cview(mhat99_i, pq),
                                         bcast(m4[0:HK, HK:2 * HK]),
                                         bcast(m4b_sb[:, 0:HK]))

            # ---- stage 2a: Gt = MhatT .* Kt (complex), 99-row/99-col blocks ----
            t1 = spool.tile([99, 12 * 99], bf16, tag="t", name="t1")
            t2 = spool.tile([99, 12 * 99], bf16, tag="t", name="t2")
            r3 = lambda ap, k: ap.rearrange("q (k p) -> q k p", k=k)
            mr_b = mhat99_r[:, 0:99].unsqueeze(1).broadcast_to([99, 12, 99])
            mi_b = mhat99_i[:, 0:99].unsqueeze(1).broadcast_to([99, 12, 99])
            # t1 = M99r * [kR;kI] on vector; t2 = M99i * [kI;kR] gpsimd/vector
            for half in range(2):
                c0, c1 = half * 594, (half + 1) * 594
                mr_b6 = mhat99_r[:, 0:99].unsqueeze(1).broadcast_to([99, 6, 99])
                mi_b6 = mhat99_i[:, 0:99].unsqueeze(1).broadcast_to([99, 6, 99])
                nc.vector.tensor_mul(r3(t1[:, c0:c1], 6), mr_b6,
                                     r3(kri_sb[:, c0:c1], 6))
                nc.vector.tensor_mul(r3(t2[:, c0:c1], 6), mi_b6,
                                     r3(kri_sb[:, 1188 + c0:1188 + c1], 6))
                nc.vector.tensor_sub(gt[0:HK, c0:c1], t1[0:HK, c0:c1], t2[0:HK, c0:c1])
                nc.vector.tensor_add(gt[64:99, c0:c1], t1[64:99, c0:c1], t2[64:99, c0:c1])

            # ---- stage 2c: W pairs; w99 = [Wr; 0; Wi] [99, 1728] ----
            # col layout: pair p low-k at 72p (0:864), high-k at 864+72p
            sq = [sqpool.tile([72, 944], bf16, tag="sq", name=f"sq{g}")
                  for g in range(4)]
            with (
                tc.tile_pool(name="wps", bufs=2, space=bass.MemorySpace.PSUM) as wps,
                tc.tile_pool(name="fps", bufs=2, space=bass.MemorySpace.PSUM) as fps,
            ):
                offs = (0, 144, 288, 512, 656, 800)    # 3 pair-slots per bank
                for t6 in range(2):                    # 6 pairs (12 kernels) / tile
                    wp = wps.tile([99, 1024], f32)
                    for j in range(6):
                        pr = t6 * 6 + j
                        nc.tensor.matmul(wp[:, offs[j]:offs[j] + 144],
                                         gt[:, pr * 99:(pr + 1) * 99],
                                         ctp99, start=True, stop=True)
                    # wp rows: 0:35 = W_lo, 64:99 = W_hi; [99, 2, 3, 144] view
                    wpv = wp[:].rearrange("q (z c) -> q z c", z=2)[
                        :, :, 0:432].rearrange("q z (j m) -> q z j m", j=3)
                    cl = slice(t6 * 432, (t6 + 1) * 432)
                    ch = slice(864 + t6 * 432, 864 + (t6 + 1) * 432)
                    r2 = lambda ap: ap.rearrange("q (z j m) -> q z j m", z=2, j=3)
                    nc.vector.tensor_copy(r2(w99[0:HK, cl]), wpv[0:HK, :, :, 0:72])
                    nc.vector.tensor_copy(r2(w99[64:99, cl]), wpv[0:HK, :, :, 72:144])
                    nc.scalar.copy(r2(w99[0:HK, ch]), wpv[64:99, :, :, 0:72])
                    nc.vector.tensor_copy(r2(w99[64:99, ch]), wpv[64:99, :, :, 72:144])

                # ---- stage 2d: F groups (6 kernels) + squares (Re/Im spans) ----
                for rnd in range(2):
                    fpa = fps.tile([72, 1024], f32, tag="fp", name="fpa")
                    fpb = fps.tile([72, 1024], f32, tag="fp", name="fpb")
                    ga, gb = 2 * rnd, 2 * rnd + 1
                    nc.tensor.matmul(fpa[:, 0:432], cc99r,
                                     w99[:, ga * 432:(ga + 1) * 432],
                                     start=True, stop=True)
                    nc.tensor.matmul(fpa[:, 512:944], cc99i,
                                     w99[:, ga * 432:(ga + 1) * 432],
                                     start=True, stop=True)
                    nc.scalar.activation(sq[ga][:], fpa[:, 0:944], AF.Square)
                    nc.tensor.matmul(fpb[:, 0:432], cc99r,
                                     w99[:, gb * 432:(gb + 1) * 432],
                                     start=True, stop=True)
                    nc.tensor.matmul(fpb[:, 512:944], cc99i,
                                     w99[:, gb * 432:(gb + 1) * 432],
                                     start=True, stop=True)
                    nc.scalar.activation(sq[gb][:], fpb[:, 0:944], AF.Square)

            # ---- intensity sum: p-chain (groups 0,1) folds early; q-chain
            # (groups 2,3) trails the last square ----
            pq_out = []
            for ch, (gx, gy) in enumerate(((0, 1), (2, 3))):
                vr = spool.tile([72, 432], bf16, tag=f"v{ch}", name=f"vr{ch}")
                vi = spool.tile([72, 432], bf16, tag=f"v{ch}", name=f"vi{ch}")
                nc.vector.tensor_add(vr[:], sq[gx][:, 0:432], sq[gy][:, 0:432])
                nc.vector.tensor_add(vi[:], sq[gx][:, 512:944], sq[gy][:, 512:944])
                fa = wpool.tile([72, 432], f32, tag=f"fa{ch}", name=f"fa{ch}")
                fb = wpool.tile([72, 216], f32, tag=f"fb{ch}", name=f"fb{ch}")
                fc = wpool.tile([72, 72], f32, tag=f"fc{ch}", name=f"fc{ch}")
                fd = wpool.tile([72, 72], bf16, tag=f"fd{ch}", name=f"fd{ch}")
                nc.vector.tensor_add(fa[:], vr[:], vi[:])
                nc.vector.tensor_add(fb[:], fa[:, 0:216], fa[:, 216:432])
                nc.vector.tensor_add(fc[:], fb[:, 0:72], fb[:, 72:144])
                nc.vector.tensor_add(fd[:], fc[:], fb[:, 144:216])
                pq_out.append(fd)

            # ---- stage 5: aerial_half = U_h @ aer_c @ U^T (bf16 matmuls) ----
            z_sb = wpool.tile([72, 512], bf16)
            with tc.tile_pool(name="zps", bufs=1, space=bass.MemorySpace.PSUM) as zps:
                zp = zps.tile([72, 512], f32)
                nc.tensor.matmul(zp[:], pq_out[0][:], uht, start=True, stop=False)
                nc.tensor.matmul(zp[:], pq_out[1][:], uht, start=False, stop=True)
                nc.scalar.copy(z_sb[:, 0:256], zp[:, 0:256])
                nc.vector.tensor_copy(z_sb[:, 256:512], zp[:, 256:512])

            # uht cols are host-permuted: z col 128*(2*tau+s)+p holds output
            # row 256*tau + 2p + s, so partition p carries 2 adjacent DRAM rows
            # per 256-row tile -> 4KB output descriptor runs
            with tc.tile_pool(name="aps", bufs=2, space=bass.MemorySpace.PSUM) as aps:
                for tau in range(2):
                    aer_sb = opool.tile([128, 2 * N], bf16, tag="out", name="aer_sb")
                    for s in range(2):
                        ap_t = aps.tile([128, N], f32)
                        zc = 256 * tau + 128 * s
                        nc.tensor.matmul(ap_t[:, 0:512],
                                         z_sb[:, zc:zc + 128],
                                         ut[:, 0:512], start=True, stop=True)
                        nc.tensor.matmul(ap_t[:, 512:1024],
                                         z_sb[:, zc:zc + 128],
                                         ut[:, 512:1024], start=True, stop=True)
                        if s == 0:
                            nc.scalar.copy(aer_sb[:, s * N:(s + 1) * N], ap_t[:])
                        else:
                            nc.vector.tensor_copy(aer_sb[:, s * N:(s + 1) * N],
                                                  ap_t[:])
                    oq = nc.sync if tau == 0 else nc.gpsimd
                    oq.dma_start(
                        aerial_d[256 * tau:256 * (tau + 1), :].rearrange(
                            "(p s) y -> p (s y)", p=128),
                        aer_sb[:])

    nc.compile()
    return nc


_CACHE = {}


def _get_program():
    if "nc" not in _CACHE:
        _CACHE["nc"] = _build_program()
    return _CACHE["nc"]


def _prep_inputs(mask, kernels, scales):
    import ml_dtypes
    bf = ml_dtypes.bfloat16

    atp, ctp99, cc99, ut, U = _host_matrices()

    kers = kernels.astype(np.complex128) * np.sqrt(scales.astype(np.float64))[:, None, None]
    ktR = np.ascontiguousarray(
        kers.real.astype(np.float32).transpose(2, 0, 1).reshape(HK, K * HK))
    ktI = np.ascontiguousarray(
        kers.imag.astype(np.float32).transpose(2, 0, 1).reshape(HK, K * HK))
    # 99-row / 99-col pair-block layout: block p holds kernels (2p, 2p+1) at
    # cols 0:35 / 64:99; rows 0:35 multiply M (kA top), rows 64:99 the swap.
    kri = np.zeros((99, 2 * 12 * 99), np.float32)
    for p in range(12):
        for side, k in ((0, 2 * p), (64, 2 * p + 1)):
            c = p * 99 + side
            kri[0:HK, c:c + HK] = ktR[:, k * HK:(k + 1) * HK]        # t1 top: Kr
            kri[64:99, c:c + HK] = ktI[:, k * HK:(k + 1) * HK]       # t1 bot: Ki
            kri[0:HK, 1188 + c:1188 + c + HK] = ktI[:, k * HK:(k + 1) * HK]
            kri[64:99, 1188 + c:1188 + c + HK] = ktR[:, k * HK:(k + 1) * HK]
    kri = kri.astype(bf)
    # atp: cols 0:560 y-interleaved (atp[8p+r, u], stage 1); cols 560:1352
    # j-chunked 99-col stacks [Ar(35) | zeros(29) | Ai(35)] (stage 1b)
    atp_y = atp.reshape(128, 8 * 2 * HK)                         # [128, 560]
    atp_j = atp.reshape(8, 128, 2 * HK).transpose(1, 0, 2)       # [128, 8, 70]
    atp99 = np.zeros((128, 8, 99), np.float32)
    atp99[:, :, 0:HK] = atp_j[:, :, 0:HK]
    atp99[:, :, 64:99] = atp_j[:, :, HK:2 * HK]
    atp = np.ascontiguousarray(
        np.concatenate([atp_y, atp99.reshape(128, 792)], axis=1))
    cc = np.concatenate([ctp99, cc99], axis=1).astype(bf)      # [99, 512]
    # uht cols permuted so stage-5b's z col 128*(2*tau+s)+p holds output row
    # 256*tau + 2p + s (2 adjacent DRAM rows per partition in the output DMA)
    cidx = np.arange(512)
    rperm = 256 * (cidx // 256) + 2 * (cidx % 128) + ((cidx % 256) // 128)
    uh = [np.ascontiguousarray(U[h * 512:(h + 1) * 512, :].T[:, rperm])
          for h in range(2)]
    uc = [np.concatenate([uh[h], ut], axis=1).astype(bf) for h in range(2)]
    atp_bf = atp.astype(bf)
    mask_bf = np.asarray(mask, np.float32).astype(bf)
    return mask_bf, atp_bf, kri, cc, uc


# ---------------------------------------------------------------- entry point
def kernel(mask, kernels, kernels_ct, scales):
    """Full inputs in, full outputs out.  Shards over 8 NeuronCores internally."""
    from concourse.bass_utils import run_bass_kernel_spmd

    kernels = np.asarray(kernels, np.complex64)
    scales = np.asarray(scales, np.float32)
    mask_bf, atp_bf, kri, cc, uc = _prep_inputs(mask, kernels, scales)

    nc = _get_program()
    in_maps = []
    for c in range(8):
        b, h = c // 2, c % 2
        in_maps.append({
            "x": mask_bf[b],
            "atp": atp_bf,
            "kri": kri,
            "cc": cc,
            "uc": uc[h],
        })

    trace = bool(int(os.environ.get("BASS_KERNEL_TRACE", "0")))
    res = run_bass_kernel_spmd(nc, in_maps, core_ids=list(range(8)), trace=trace)
    _CACHE["last_results"] = res

    aerial = np.empty((B, N, N), np.float32)
    for c in range(8):
        b, h = c // 2, c % 2
        aerial[b, h * 512:(h + 1) * 512, :] = \
            np.asarray(res.results[c]["aerial"]).astype(np.float32)
    resist = (1.0 / (1.0 + np.exp(
        -RESIST_STEEPNESS * (aerial.astype(np.float64) - RESIST_THRESHOLD)
    ))).astype(np.float32)
    printed = (aerial > RESIST_THRESHOLD).astype(np.float32)
    return aerial, resist, printed
